# revision 13
# baseline (speedup 1.0000x reference)
"""Trainium2 Bass kernel for nn_Analytic_net (gnn_message_passing).

Computes: A = constant_part + einsum('eij,e->ij', M, r);
          out = solve(A, [zeros(500, P); x.reshape(12, P)])   # (512, 4096)

Distribution (8 NeuronCores): M pre-scaled by r on the host, cast fp16,
sharded along E (128 edges/core).  Each core sums its 128 edge matrices
on the TensorEngine (identity-matmul PSUM accumulation over fp16 tiles,
DMA-bound), adds constant_part/8, AllReduces the (512,512) partial.

Replicated solve (every core): ridge-SPD preconditioned CG --
  G = A@A^T + eta*I  (SPD, lambda_min >= eta)
  block Cholesky-LU of G (4x128 blocks); each diagonal block inverted
  by a drift-free gamma-capped Newton-Schulz iteration (all fp32);
  outer CG on V = A A^T Shat(.)  (symmetric PSD, clustered spectrum)
  for the 12 rhs columns; x = A^T Shat(z); out_shard = x @ e_shard.

Self-contained: hardcodes all shapes; builds host-side constants inline.
"""
import numpy as np

NCORES = 8
E, N, MD, NPTS = 1024, 512, 12, 4096
ESH = E // NCORES            # 128 edges per core
JSH = NPTS // NCORES         # 512 points per core
NB = N // 128                # 4 blocks of 128
BS = 128
W48 = 4 * MD                 # 48

ETA_REL = 4e-5
MARGIN = 1.5625
NS_L0 = 2e-6
NS_POLISH = 3
GCAP = 1.9
CG_ITERS = 20

_F = np.float32


def _schedule(l0=NS_L0, polish=NS_POLISH):
    l = l0; gs = []
    while l < 0.7:
        g = min(2.0 / (1.0 + l), GCAP)
        gs.append(g)
        l = min(g * l * (2.0 - g * l), g * (2.0 - g))
    return gs + [1.0] * polish


def _build(dbg=False, reps=1):
    import concourse.bacc as bacc
    import concourse.tile as tile
    import concourse.mybir as mybir

    dt = mybir.dt.float32
    f16 = mybir.dt.float16
    nc = bacc.Bacc("TRN2", target_bir_lowering=False, debug=False,
                   num_devices=NCORES)

    M_d = nc.dram_tensor("M", [64 * BS, 4096], f16, kind="ExternalInput")
    C8_d = nc.dram_tensor("C8", [64, 4096], dt, kind="ExternalInput")
    e_d = nc.dram_tensor("e", [MD, JSH], dt, kind="ExternalInput")
    I16_d = nc.dram_tensor("I16", [BS, BS], f16, kind="ExternalInput")
    ID_d = nc.dram_tensor("ID", [BS, BS], dt, kind="ExternalInput")
    I2_d = nc.dram_tensor("I2", [BS, BS], dt, kind="ExternalInput")
    b48_d = nc.dram_tensor("b48", [BS, W48], dt, kind="ExternalInput")
    out_d = nc.dram_tensor("out", [N, JSH], dt, kind="ExternalOutput")
    dbg_d = {}
    if dbg:
        for nm, shp in [("dbg_A", [BS, 2048]), ("dbg_G", [BS, 2048]),
                        ("dbg_Gw", [BS, 2048]), ("dbg_D0", [BS, BS]),
                        ("dbg_D3", [BS, BS]), ("dbg_W", [BS, W48]),
                        ("dbg_Z", [BS, W48])]:
            dbg_d[nm] = nc.dram_tensor(nm, shp, dt, kind="ExternalOutput")

    add = mybir.AluOpType.add
    sub = mybir.AluOpType.subtract
    mult = mybir.AluOpType.mult
    AF = mybir.ActivationFunctionType

    gammas = _schedule()

    def gblk(i, j):
        """Column slice of a (128,2048) block-layout tile for block (i,j)."""
        return slice((i * NB + j) * BS, (i * NB + j + 1) * BS)

    with tile.TileContext(nc) as tc:
        with tc.tile_pool(name="dram", bufs=(2 if reps > 1 else 1),
                          space="DRAM") as dram, \
             tc.tile_pool(name="consts", bufs=1) as cp, \
             tc.tile_pool(name="mats", bufs=1) as mats, \
             tc.tile_pool(name="fact", bufs=1) as fp, \
             tc.tile_pool(name="ns", bufs=2) as nsp, \
             tc.tile_pool(name="work", bufs=2) as work, \
             tc.tile_pool(name="small", bufs=2) as sp:

            # ------------- consts -------------
            I16_sb = cp.tile([BS, BS], f16)
            ID_sb = cp.tile([BS, BS], dt)
            I2_sb = cp.tile([BS, BS], dt)
            b48_sb = cp.tile([BS, W48], dt)
            C8_sb = cp.tile([64, 4096], dt)
            e_sb = cp.tile([MD, JSH], dt)
            ones_col = cp.tile([BS, 1], dt)
            ones_row = cp.tile([1, BS], dt)
            nc.sync.dma_start(I16_sb[:], I16_d[:])
            nc.sync.dma_start(ID_sb[:], ID_d[:])
            nc.sync.dma_start(I2_sb[:], I2_d[:])
            nc.sync.dma_start(b48_sb[:], b48_d[:])
            nc.sync.dma_start(C8_sb[:], C8_d[:])
            nc.sync.dma_start(e_sb[:], e_d[:])
            nc.vector.memset(ones_col[:], 1.0)
            nc.vector.memset(ones_row[:], 1.0)

            for _rep in range(reps):
                # ------------- Phase E: einsum via PE psum accumulate ------
                partial_b = dram.tile([N, N], dt, tag="partial")
                ar_b = dram.tile([N, N], dt, tag="ar")
                pb_r = partial_b.rearrange("(p a) j -> p (a j)", a=8)
                with tc.tile_pool(name="epsum", bufs=1, space="PSUM") as ep, \
                     tc.tile_pool(name="mtiles", bufs=3) as mp:
                    eps = [ep.tile([BS, 512], dt, tag=f"eb{ci}",
                                   name=f"eb{ci}")
                           for ci in range(8)]
                    for f in range(64):
                        mt = mp.tile([BS, 4096], f16, tag="mt")
                        nc.sync.dma_start(mt[:], M_d[BS * f:BS * (f + 1), :])
                        for ci in range(8):
                            nc.tensor.matmul(
                                eps[ci][:], lhsT=I16_sb[:],
                                rhs=mt[:, 512 * ci:512 * (ci + 1)],
                                start=(f == 0), stop=(f == 63))
                    part = work.tile([64, 4096], dt, tag="part")
                    for ci in range(8):
                        sl = slice(512 * ci, 512 * (ci + 1))
                        nc.vector.tensor_tensor(
                            part[:, sl], eps[ci][0:64, :], C8_sb[:, sl],
                            op=add)
                        nc.vector.tensor_tensor(
                            part[:, sl], part[:, sl], eps[ci][64:BS, :],
                            op=add)
                    nc.sync.dma_start(pb_r[:, :], part[:])

                # ------------- Phase R: AllReduce -------------
                nc.gpsimd.collective_compute(
                    "AllReduce", add, replica_groups=[list(range(NCORES))],
                    ins=[partial_b.opt()], outs=[ar_b.opt()])

                # ------------- Phase A: load A (block layout) -------------
                A_sb = mats.tile([BS, 2048], dt, tag="A")
                for bi in range(NB):
                    nc.sync.dma_start(A_sb[:, bi * N:(bi + 1) * N],
                                      ar_b[BS * bi:BS * (bi + 1), :])

                with tc.tile_pool(name="psumA", bufs=2, space="PSUM") as pp, \
                     tc.tile_pool(name="psumB", bufs=2, space="PSUM") as ppb, \
                     tc.tile_pool(name="psumN", bufs=2, space="PSUM") as ppn:

                    # ---------- Phase T: At (16 PE transposes) ----------
                    At_sb = mats.tile([BS, 2048], dt, tag="At")
                    for bj in range(NB):
                        ptr = ppb.tile([BS, N], dt, tag="ps_small")
                        for bi in range(NB):
                            nc.tensor.transpose(
                                ptr[:, bi * BS:(bi + 1) * BS],
                                A_sb[:, gblk(bi, bj)], ID_sb[:])
                        nc.scalar.copy(At_sb[:, bj * N:(bj + 1) * N], ptr[:])

                    # ---------- Phase N: eta from ||A||_F^2 ----------
                    junk = work.tile([BS, 2048], dt, tag="junk")
                    rowsq = sp.tile([BS, 1], dt, tag="rowsq")
                    nc.vector.scalar_tensor_tensor(
                        junk[:], A_sb[:], 1.0, A_sb[:], op0=mult, op1=mult,
                        accum_out=rowsq[:])
                    pfro = ppb.tile([BS, 512], dt, tag="ps_small")
                    nc.tensor.matmul(pfro[0:1, 0:1], lhsT=ones_col[:],
                                     rhs=rowsq[:], start=True, stop=True)
                    eta_s = sp.tile([1, 1], dt, tag="eta_s")
                    nc.scalar.activation(
                        eta_s[:], pfro[0:1, 0:1], AF.Copy,
                        scale=float(ETA_REL * MARGIN * 4.0 / N))
                    pbc = ppb.tile([BS, 512], dt, tag="ps_small")
                    nc.tensor.matmul(pbc[:, 0:1], lhsT=ones_row[:],
                                     rhs=eta_s[:], start=True, stop=True)
                    eta_bc = sp.tile([BS, 1], dt, tag="eta_bc")
                    nc.scalar.copy(eta_bc[:], pbc[:, 0:1])

                    # ---------- Phase G: G = A@A^T + eta*I ----------
                    G_sb = mats.tile([BS, 2048], dt, tag="G")
                    for m in range(NB):
                        pm = pp.tile([BS, N], dt, tag="pmm")
                        for k in range(NB):
                            nc.tensor.matmul(
                                pm[:], lhsT=At_sb[:, gblk(k, m)],
                                rhs=At_sb[:, k * N:(k + 1) * N],
                                start=(k == 0), stop=(k == NB - 1))
                        nc.scalar.copy(G_sb[:, m * N:(m + 1) * N], pm[:])
                    for k in range(NB):
                        nc.vector.scalar_tensor_tensor(
                            G_sb[:, gblk(k, k)], ID_sb[:], eta_bc[:],
                            G_sb[:, gblk(k, k)], op0=mult, op1=add)

                    # ---------- Phase F: block Cholesky-LU ----------
                    Dinv = []
                    Lt = {}
                    for k in range(NB):
                        # c = 1/frobenius(D)
                        dj = nsp.tile([BS, BS], dt, tag="dj")
                        dsq = sp.tile([BS, 1], dt, tag="dsq")
                        nc.vector.scalar_tensor_tensor(
                            dj[:], G_sb[:, gblk(k, k)], 1.0,
                            G_sb[:, gblk(k, k)], op0=mult, op1=mult,
                            accum_out=dsq[:])
                        pfr = ppb.tile([BS, 512], dt, tag="ps_small")
                        nc.tensor.matmul(pfr[0:1, 0:1], lhsT=ones_col[:],
                                         rhs=dsq[:], start=True, stop=True)
                        fr_s = sp.tile([1, 1], dt, tag="fr_s")
                        nc.scalar.activation(fr_s[:], pfr[0:1, 0:1], AF.Sqrt)
                        c_s = sp.tile([1, 1], dt, tag="c_s")
                        nc.vector.reciprocal(c_s[:], fr_s[:])
                        pcb = ppb.tile([BS, 512], dt, tag="ps_small")
                        nc.tensor.matmul(pcb[:, 0:1], lhsT=ones_row[:],
                                         rhs=c_s[:], start=True, stop=True)
                        c_bc = sp.tile([BS, 1], dt, tag="c_bc")
                        nc.scalar.copy(c_bc[:], pcb[:, 0:1])

                        # B = c*D ; X = I
                        B_t = nsp.tile([BS, BS], dt, tag="B")
                        nc.vector.tensor_scalar_mul(
                            B_t[:], G_sb[:, gblk(k, k)], c_bc[:])
                        X_t = nsp.tile([BS, BS], dt, tag="X")
                        nc.vector.tensor_copy(X_t[:], ID_sb[:])
                        n_ns = len(gammas)
                        for gi, g in enumerate(gammas):
                            gf = float(g)
                            pt = ppn.tile([BS, BS], dt, tag="nsT")
                            nc.tensor.matmul(pt[:], lhsT=B_t[:], rhs=X_t[:],
                                             start=True, stop=True)
                            P_t = nsp.tile([BS, BS], dt, tag="P")
                            nc.vector.scalar_tensor_tensor(
                                P_t[:], pt[:], -gf, I2_sb[:],
                                op0=mult, op1=add)
                            px = ppn.tile([BS, BS], dt, tag="nsX")
                            nc.tensor.matmul(px[:], lhsT=X_t[:], rhs=P_t[:],
                                             start=True, stop=True)
                            Xn = nsp.tile([BS, BS], dt, tag="X")
                            nc.scalar.activation(Xn[:], px[:], AF.Copy,
                                                 scale=gf)
                            # X drifts asymmetric (matmul computes X^T P and
                            # fp rounding of BX is asymmetric); the NS map
                            # amplifies it 4x/iter -- re-symmetrize.
                            if gi % 2 == 1 or gi == n_ns - 1:
                                ptx = ppn.tile([BS, BS], dt, tag="nsT")
                                nc.tensor.transpose(ptx[:], Xn[:], ID_sb[:])
                                Xh = nsp.tile([BS, BS], dt, tag="P")
                                nc.scalar.activation(Xh[:], ptx[:], AF.Copy,
                                                     scale=0.5)
                                Xs = nsp.tile([BS, BS], dt, tag="X")
                                nc.vector.scalar_tensor_tensor(
                                    Xs[:], Xn[:], 0.5, Xh[:],
                                    op0=mult, op1=add)
                                Xn = Xs
                            X_t = Xn
                        Dk = fp.tile([BS, BS], dt, tag=f"dinv{k}")
                        nc.vector.tensor_scalar_mul(Dk[:], X_t[:], c_bc[:])
                        Dinv.append(Dk)

                        # Lt(k,i) = Dinv_k @ G(k,i) ;  Schur update row-span
                        for i in range(k + 1, NB):
                            pl = ppb.tile([BS, 512], dt, tag="ps_small")
                            nc.tensor.matmul(pl[:, 0:BS], lhsT=Dk[:],
                                             rhs=G_sb[:, gblk(k, i)],
                                             start=True, stop=True)
                            lt = fp.tile([BS, BS], dt, tag=f"lt{k}{i}")
                            nc.scalar.copy(lt[:], pl[:, 0:BS])
                            Lt[(k, i)] = lt
                        span = slice((k * NB + k + 1) * BS, (k * NB + NB) * BS)
                        wsp = (NB - 1 - k) * BS
                        for i in range(k + 1, NB):
                            ps = pp.tile([BS, N], dt, tag="pmm")
                            nc.tensor.matmul(ps[:, 0:wsp], lhsT=Lt[(k, i)][:],
                                             rhs=G_sb[:, span],
                                             start=True, stop=True)
                            ispan = slice((i * NB + k + 1) * BS,
                                          (i * NB + NB) * BS)
                            nc.vector.tensor_tensor(
                                G_sb[:, ispan], G_sb[:, ispan], ps[:, 0:wsp],
                                op=sub)

                    # ---------- Phase H: H = Shat(I); K = A^T H; Kt ----------
                    ys = work.tile([BS, 2048], dt, tag="ys")
                    nc.vector.memset(ys[:], 0.0)
                    for i in range(NB):
                        nc.vector.tensor_copy(
                            ys[:, i * N + i * BS:i * N + (i + 1) * BS],
                            ID_sb[:])
                    for i in range(1, NB):
                        pf = pp.tile([BS, N], dt, tag="pmm")
                        for k in range(i):
                            nc.tensor.matmul(
                                pf[:], lhsT=Lt[(k, i)][:],
                                rhs=ys[:, k * N:(k + 1) * N],
                                start=(k == 0), stop=(k == i - 1))
                        nc.vector.tensor_tensor(
                            ys[:, i * N:(i + 1) * N],
                            ys[:, i * N:(i + 1) * N], pf[:], op=sub)
                    H_sb = mats.tile([BS, 2048], dt, tag="H")
                    for i in reversed(range(NB)):
                        if i < NB - 1:
                            pb2 = pp.tile([BS, N], dt, tag="pmm")
                            for j in range(i + 1, NB):
                                nc.tensor.matmul(
                                    pb2[:], lhsT=G_sb[:, gblk(j, i)],
                                    rhs=H_sb[:, j * N:(j + 1) * N],
                                    start=(j == i + 1), stop=(j == NB - 1))
                            t512 = work.tile([BS, N], dt, tag="t512")
                            nc.vector.scalar_tensor_tensor(
                                t512[:], pb2[:], -1.0,
                                ys[:, i * N:(i + 1) * N], op0=mult, op1=add)
                            rhs_t = t512[:]
                        else:
                            rhs_t = ys[:, i * N:(i + 1) * N]
                        pd = pp.tile([BS, N], dt, tag="pmm")
                        nc.tensor.matmul(pd[:], lhsT=Dinv[i][:], rhs=rhs_t,
                                         start=True, stop=True)
                        nc.scalar.copy(H_sb[:, i * N:(i + 1) * N], pd[:])
                    K_sb = mats.tile([BS, 2048], dt, tag="K")
                    for m in range(NB):
                        pk = pp.tile([BS, N], dt, tag="pmm")
                        for k in range(NB):
                            nc.tensor.matmul(
                                pk[:], lhsT=A_sb[:, gblk(k, m)],
                                rhs=H_sb[:, k * N:(k + 1) * N],
                                start=(k == 0), stop=(k == NB - 1))
                        nc.scalar.copy(K_sb[:, m * N:(m + 1) * N], pk[:])
                    # A_sb is dead from here on -- reuse its buffer for Kt
                    Kt_sb = mats.tile([BS, 2048], dt, tag="A")
                    for bj in range(NB):
                        ptk = ppb.tile([BS, N], dt, tag="ps_small")
                        for bi in range(NB):
                            nc.tensor.transpose(
                                ptk[:, bi * BS:(bi + 1) * BS],
                                K_sb[:, gblk(bi, bj)], ID_sb[:])
                        nc.scalar.copy(Kt_sb[:, bj * N:(bj + 1) * N], ptk[:])

                    def apply_mat(lhsT_mat, src, outtag):
                        """M @ src where lhsT_mat holds M^T blocks."""
                        pm = ppb.tile([BS, 512], dt, tag="ps_small")
                        for m in range(NB):
                            for k in range(NB):
                                nc.tensor.matmul(
                                    pm[:, m * MD:(m + 1) * MD],
                                    lhsT=lhsT_mat[:, gblk(k, m)],
                                    rhs=src[:, k * MD:(k + 1) * MD],
                                    start=(k == 0), stop=(k == NB - 1))
                        o = sp.tile([BS, W48], dt, tag=outtag)
                        nc.scalar.copy(o[:], pm[:, 0:W48])
                        return o

                    def preduce(src48, dst12, tagbase):
                        pr = ppb.tile([BS, 512], dt, tag="ps_small")
                        nc.tensor.matmul(pr[0:1, 0:W48], lhsT=ones_col[:],
                                         rhs=src48[:], start=True, stop=True)
                        d48 = sp.tile([1, W48], dt, tag="d48_" + tagbase)
                        nc.scalar.copy(d48[:], pr[0:1, 0:W48])
                        nc.vector.tensor_tensor(d48[:, 0:MD], d48[:, 0:MD],
                                                d48[:, MD:2 * MD], op=add)
                        nc.vector.tensor_tensor(d48[:, 2 * MD:3 * MD],
                                                d48[:, 2 * MD:3 * MD],
                                                d48[:, 3 * MD:4 * MD], op=add)
                        nc.vector.tensor_tensor(dst12[:], d48[:, 0:MD],
                                                d48[:, 2 * MD:3 * MD], op=add)

                    def bcast12(src12, dst48):
                        a48 = sp.tile([1, W48], dt, tag="a48")
                        for kk in range(NB):
                            nc.vector.tensor_copy(
                                a48[:, kk * MD:(kk + 1) * MD], src12[:])
                        pb3 = ppb.tile([BS, 512], dt, tag="ps_small")
                        nc.tensor.matmul(pb3[:, 0:W48], lhsT=ones_row[:],
                                         rhs=a48[:], start=True, stop=True)
                        nc.scalar.copy(dst48[:], pb3[:, 0:W48])

                    # ---------- Phase CG ----------
                    Z_t = sp.tile([BS, W48], dt, tag="Z")
                    R_t = sp.tile([BS, W48], dt, tag="R")
                    Pc_t = sp.tile([BS, W48], dt, tag="Pc")
                    rs_t = sp.tile([1, MD], dt, tag="rs")
                    tt = sp.tile([BS, W48], dt, tag="tt")
                    nc.vector.memset(Z_t[:], 0.0)
                    nc.vector.tensor_copy(R_t[:], b48_sb[:])
                    nc.vector.tensor_copy(Pc_t[:], b48_sb[:])
                    nc.vector.tensor_tensor(tt[:], R_t[:], R_t[:], op=mult)
                    preduce(tt, rs_t, "rs")

                    for it in range(CG_ITERS):
                        u2 = apply_mat(Kt_sb, Pc_t, "u2")
                        Vp = apply_mat(At_sb, u2, "Vp")
                        nc.vector.tensor_tensor(tt[:], Pc_t[:], Vp[:],
                                                op=mult)
                        den = sp.tile([1, MD], dt, tag="den")
                        preduce(tt, den, "den")
                        rd = sp.tile([1, MD], dt, tag="rd")
                        nc.vector.reciprocal(rd[:], den[:])
                        alpha = sp.tile([1, MD], dt, tag="alpha")
                        nc.vector.tensor_tensor(alpha[:], rs_t[:], rd[:],
                                                op=mult)
                        ab = sp.tile([BS, W48], dt, tag="ab")
                        bcast12(alpha, ab)
                        nc.vector.tensor_tensor(tt[:], ab[:], Pc_t[:],
                                                op=mult)
                        nc.vector.tensor_tensor(Z_t[:], Z_t[:], tt[:], op=add)
                        nc.vector.tensor_tensor(tt[:], ab[:], Vp[:], op=mult)
                        nc.vector.tensor_tensor(R_t[:], R_t[:], tt[:], op=sub)
                        if it == CG_ITERS - 1:
                            break
                        nc.vector.tensor_tensor(tt[:], R_t[:], R_t[:],
                                                op=mult)
                        rsn = sp.tile([1, MD], dt, tag="rsn")
                        preduce(tt, rsn, "rsn")
                        rr = sp.tile([1, MD], dt, tag="rr")
                        nc.vector.reciprocal(rr[:], rs_t[:])
                        beta = sp.tile([1, MD], dt, tag="beta")
                        nc.vector.tensor_tensor(beta[:], rsn[:], rr[:],
                                                op=mult)
                        nc.vector.tensor_copy(rs_t[:], rsn[:])
                        bb = sp.tile([BS, W48], dt, tag="bb")
                        bcast12(beta, bb)
                        nc.vector.tensor_tensor(tt[:], bb[:], Pc_t[:],
                                                op=mult)
                        nc.vector.tensor_tensor(Pc_t[:], R_t[:], tt[:],
                                                op=add)

                    # ---------- Phase X: W = K @ z ----------
                    W_t = apply_mat(Kt_sb, Z_t, "W")

                    if dbg:
                        nc.sync.dma_start(dbg_d["dbg_A"][:], A_sb[:])
                        nc.sync.dma_start(dbg_d["dbg_G"][:], G_sb[:])
                        nc.sync.dma_start(dbg_d["dbg_Gw"][:], G_sb[:])
                        nc.sync.dma_start(dbg_d["dbg_D0"][:], Dinv[0][:])
                        nc.sync.dma_start(dbg_d["dbg_D3"][:], Dinv[3][:])
                        nc.sync.dma_start(dbg_d["dbg_W"][:], W_t[:])
                        nc.sync.dma_start(dbg_d["dbg_Z"][:], Z_t[:])

                    # ---------- Phase OUT: out = W-as-(512,12) @ e ----------
                    Wt_sb = sp.tile([MD, N], dt, tag="Wt")
                    pyt = ppb.tile([BS, 512], dt, tag="ps_small")
                    for m in range(NB):
                        nc.tensor.transpose(pyt[0:MD, m * BS:(m + 1) * BS],
                                            W_t[:, m * MD:(m + 1) * MD],
                                            ID_sb[:])
                    nc.scalar.copy(Wt_sb[:], pyt[0:MD, :])
                    for mi in range(NB):
                        po = pp.tile([BS, N], dt, tag="pmm")
                        nc.tensor.matmul(po[:, 0:JSH],
                                         lhsT=Wt_sb[:, mi * BS:(mi + 1) * BS],
                                         rhs=e_sb[:], start=True, stop=True)
                        ot = work.tile([BS, JSH], dt, tag="ot")
                        nc.scalar.copy(ot[:], po[:, 0:JSH])
                        nc.sync.dma_start(out_d[mi * BS:(mi + 1) * BS, :],
                                          ot[:])

    nc.compile()
    return nc


_NC_CACHE = {}


def _get_nc(key=(False, 1)):
    if key not in _NC_CACHE:
        _NC_CACHE[key] = _build(dbg=key[0], reps=key[1])
    return _NC_CACHE[key]


def make_in_maps(M, r, constant_part, x):
    Ms = (np.asarray(M, dtype=_F)
          * np.asarray(r, dtype=_F)[:, None, None]).astype(np.float16)
    C8 = (np.ascontiguousarray(constant_part, dtype=_F)
          / _F(NCORES)).reshape(64, 4096)
    e_full = np.ascontiguousarray(x, dtype=_F).reshape(MD, NPTS)
    I16 = np.eye(BS, dtype=np.float16)
    ID = np.eye(BS, dtype=_F)
    I2 = (2.0 * np.eye(BS, dtype=_F)).astype(_F)
    bfull = np.zeros((N, MD), dtype=_F)
    bfull[N - MD:, :] = np.eye(MD, dtype=_F)
    b48 = np.ascontiguousarray(
        bfull.reshape(NB, BS, MD).transpose(1, 0, 2).reshape(BS, W48))
    in_maps = []
    for c in range(NCORES):
        in_maps.append({
            "M": np.ascontiguousarray(
                Ms[c * ESH:(c + 1) * ESH]).reshape(64 * BS, 4096),
            "C8": C8,
            "e": np.ascontiguousarray(e_full[:, c * JSH:(c + 1) * JSH]),
            "I16": I16, "ID": ID, "I2": I2, "b48": b48,
        })
    return in_maps


def kernel(M, r, constant_part, x):
    from concourse.bass_utils import run_bass_kernel_spmd
    nc = _get_nc()
    in_maps = make_in_maps(M, r, constant_part, x)
    res = run_bass_kernel_spmd(nc, in_maps, core_ids=list(range(NCORES)))
    out = np.concatenate([res.results[c]["out"] for c in range(NCORES)],
                         axis=1)
    return np.ascontiguousarray(out, dtype=_F)
